# revision 16
# baseline (speedup 1.0000x reference)
"""Trainium2 Bass kernel for capsule routing (nn_Capsule).

Reference computation:
    u_hat = einsum('bic,ce->bie', u_vecs, W).reshape(B, I, N, D).transpose(0,2,1,3)
    b = 0
    for r in range(3):
        c = softmax(b, axis=1)                      # over capsules n
        out = squash(einsum('bni,bnid->bnd', c, u_hat))
        if r < 2: b = einsum('bnd,bnid->bni', out, u_hat)
    return out    # (B, N, D)

Key algebraic restructuring (u_hat is never materialized; it is 32 MiB per
core and every use of it factors through u_vecs and W):
    round 0:  c uniform = 1/N  ->  out0 = squash((1/N) * (sum_i u[b,i,:]) @ W)
    logits[b,i,n] = sum_c u[b,i,c] * V[b,c,n],   V[b,c,n] = sum_d W[c,(n,d)] o[b,n,d]
    T[b,n,c]     = sum_i softmax(logits)[b,i,n] * u[b,i,c]
    pre[b,n,d]   = sum_c T[b,n,c] * W[c,(n,d)]   -> out = squash(pre)

pre is computed as the dense product T @ W (big 512-col matmuls) followed by
32 small diagonal-block extractions into a ((n,b), d) layout where squash is
pure free-dim DVE/ACT work.  rsqrt is exp(-0.5*ln(x+eps)) so the whole kernel
uses one ACT table set (natural_log_exp_and_others).

Sharding: data-parallel over batch, 4 batches per core x 8 cores, W replicated.
"""

import numpy as np
from contextlib import ExitStack

import concourse.bass as bass
import concourse.bacc as bacc
import concourse.tile as tile
from concourse import mybir
from concourse.bass_utils import run_bass_kernel_spmd
from concourse.masks import make_identity

B, I, C = 32, 1024, 256
N, D = 32, 64
ND = N * D
ROUTINGS = 3
EPS = 1e-7
NCORES = 8
BL = B // NCORES  # batches per core
IC = I // 128     # i chunks of 128
CK = C // 128     # c chunks of 128
NB = N * BL       # 128 = (n, b) composite
F32 = mybir.dt.float32
MULT = mybir.AluOpType.mult
AF = mybir.ActivationFunctionType


def _capsule_body(ctx: ExitStack, tc: tile.TileContext, out_ap, u_ap, w_ap):
    nc = tc.nc

    const = ctx.enter_context(tc.tile_pool(name="const", bufs=1))
    persist = ctx.enter_context(tc.tile_pool(name="persist", bufs=1))
    work = ctx.enter_context(tc.tile_pool(name="work", bufs=2))

    # ---- constants ----
    ident = const.tile([128, 128], F32)
    make_identity(nc, ident[:])
    eps1 = const.tile([1, 1], F32)
    nc.gpsimd.memset(eps1[:], EPS)
    ones_col = const.tile([128, 1], F32)
    nc.vector.memset(ones_col[:], 1.0)
    ones_row = const.tile([1, 64], F32)
    nc.gpsimd.memset(ones_row[:], 1.0)
    negln_n = const.tile([1, 1], F32)
    nc.gpsimd.memset(negln_n[:], float(-np.log(N)))
    zero1 = const.tile([1, 1], F32)
    nc.gpsimd.memset(zero1[:], 0.0)

    # ---- persistent SBUF tensors ----
    w_sb = persist.tile([128, CK, ND], F32)       # [q, ck, (n,d)]
    wt_sb = persist.tile([64, N, C], F32)         # [d, n, c]
    u_sb = persist.tile([128, BL, IC, C], F32)    # [p, b, ic, c]
    ut_sb = persist.tile([128, BL, CK, I], F32)   # [q, b, ck, i]
    st_sb = persist.tile([128, CK, BL], F32)      # [q, ck, b]  (column sums of u)

    # ---- load inputs ----
    for ck in range(CK):
        nc.sync.dma_start(out=w_sb[:, ck, :], in_=w_ap[ck * 128:(ck + 1) * 128, :])
    for b in range(BL):
        for ic in range(IC):
            nc.sync.dma_start(
                out=u_sb[:, b, ic, :],
                in_=u_ap[b, ic * 128:(ic + 1) * 128, :],
            )

    # ---- setup transposes (PE) ----
    with tc.tile_pool(name="ps_setup", bufs=2, space="PSUM") as ps_setup, \
            nc.named_scope("setup"):
        # W blocks:  wt[d, n, ck*128:+128] = W[ck-chunk, n-block].T
        for ck in range(CK):
            for n in range(N):
                wt_ps = ps_setup.tile([64, 128], F32, tag="wt")
                nc.tensor.transpose(
                    wt_ps[:], w_sb[:, ck, n * 64:(n + 1) * 64], ident[:]
                )
                if n % 2 == 0:
                    nc.vector.tensor_copy(
                        out=wt_sb[0:64, n, ck * 128:(ck + 1) * 128], in_=wt_ps[:]
                    )
                else:
                    nc.scalar.copy(
                        out=wt_sb[0:64, n, ck * 128:(ck + 1) * 128], in_=wt_ps[:]
                    )
        # u blocks: ut[q, b, ck, ic*128:+128] = u[b, i-chunk, c-chunk].T
        for b in range(BL):
            for ck in range(CK):
                for ic in range(IC):
                    ut_ps = ps_setup.tile([128, 128], F32, tag="ut")
                    nc.tensor.transpose(
                        ut_ps[:], u_sb[:, b, ic, ck * 128:(ck + 1) * 128], ident[:]
                    )
                    if (ic + ck) % 2 == 0:
                        nc.vector.tensor_copy(
                            out=ut_sb[:, b, ck, ic * 128:(ic + 1) * 128], in_=ut_ps[:]
                        )
                    else:
                        nc.scalar.copy(
                            out=ut_sb[:, b, ck, ic * 128:(ic + 1) * 128], in_=ut_ps[:]
                        )
        # column sums of u: st[q, ck, b] = sum_i u[b, i, ck-chunk]
        for b in range(BL):
            for ck in range(CK):
                nc.vector.reduce_sum(
                    out=st_sb[:, ck, b:b + 1],
                    in_=ut_sb[:, b, ck, :],
                    axis=mybir.AxisListType.X,
                )

    ps = ctx.enter_context(tc.tile_pool(name="ps_main", bufs=1, space="PSUM"))
    ps_pre = ctx.enter_context(tc.tile_pool(name="ps_pre", bufs=2, space="PSUM"))

    o_sb = None
    for r in range(ROUTINGS):
        if r > 0:
            # V[b][c, n] = sum_d W[c,(n,d)] o[b,n,d]
            with nc.named_scope(f"r{r}_v"):
                v_ps = ps.tile([128, CK, N, BL], F32, tag="v")
                for n in range(N):
                    for ck in range(CK):
                        nc.tensor.matmul(
                            out=v_ps[:, ck, n, :],
                            lhsT=wt_sb[0:64, n, ck * 128:(ck + 1) * 128],
                            rhs=o_sb[:, n * BL:(n + 1) * BL],
                            start=True,
                            stop=True,
                        )
                v_sb = work.tile([128, CK, N, BL], F32, tag="v_sb")
                nc.scalar.copy(out=v_sb[:], in_=v_ps[:])

            # logits[b][i, n] = sum_c u[b,i,c] V[b][c,n]   (all b in one tile)
            with nc.named_scope(f"r{r}_lg"):
                lg_ps = ps.tile([128, BL, IC, N], F32, tag="lg")
                for b in range(BL):
                    for ic in range(IC):
                        for ck in range(CK):
                            nc.tensor.matmul(
                                out=lg_ps[:, b, ic, :],
                                lhsT=ut_sb[:, b, ck, ic * 128:(ic + 1) * 128],
                                rhs=v_sb[:, ck, :, b],
                                start=(ck == 0),
                                stop=(ck == CK - 1),
                            )

            # softmax over n (free dim; no max-subtraction needed, logits O(1))
            with nc.named_scope(f"r{r}_sm"):
                e_sb = work.tile([128, BL, IC, N], F32, tag="e")
                nc.scalar.activation(out=e_sb[:], in_=lg_ps[:], func=AF.Exp)
                s_sb = work.tile([128, BL, IC], F32, tag="s")
                nc.vector.reduce_sum(
                    out=s_sb[:], in_=e_sb[:], axis=mybir.AxisListType.X
                )
                sr_sb = work.tile([128, BL, IC], F32, tag="sr")
                nc.vector.reciprocal(out=sr_sb[:], in_=s_sb[:])
                c_sb = work.tile([128, BL, IC, N], F32, tag="c")
                nc.vector.tensor_tensor(
                    c_sb[:],
                    e_sb[:],
                    sr_sb[:, :, :, None].to_broadcast([128, BL, IC, N]),
                    MULT,
                )

            # T[b][n, c] = sum_i c[i, n] u[b, i, c];  transpose into
            # tt[q, ck, n, b] (n-major columns)
            with nc.named_scope(f"r{r}_t"):
                tt_ps = ps.tile([128, CK, N, BL], F32, tag="tt")
                for b in range(BL):
                    t_ps = ps.tile([32, C], F32, tag="t")
                    for ic in range(IC):
                        nc.tensor.matmul(
                            out=t_ps[:],
                            lhsT=c_sb[:, b, ic, :],
                            rhs=u_sb[:, b, ic, :],
                            start=(ic == 0),
                            stop=(ic == IC - 1),
                        )
                    t_sb = work.tile([32, C], F32, tag="t_sb")
                    if b % 2 == 0:
                        nc.scalar.copy(out=t_sb[:], in_=t_ps[:])
                    else:
                        nc.vector.tensor_copy(out=t_sb[:], in_=t_ps[:])
                    for ck in range(CK):
                        nc.tensor.transpose(
                            tt_ps[:, ck, :, b],
                            t_sb[:, ck * 128:(ck + 1) * 128],
                            ident[0:32, 0:32],
                        )
                tt_sb = work.tile([128, CK, N, BL], F32, tag="tt_sb")
                nc.vector.tensor_copy(out=tt_sb[:], in_=tt_ps[:])

        # ---------- pre[d, (n,b)] per-capsule: pre_n = W_n.T @ T_n ----------
        with nc.named_scope(f"r{r}_pre"):
            pre_ps = ps_pre.tile([64, N, BL], F32, tag="pre")
            for n in range(N):
                for ck in range(CK):
                    rhs = (
                        st_sb[:, ck, :] if r == 0 else tt_sb[:, ck, n, :]
                    )
                    nc.tensor.matmul(
                        out=pre_ps[:, n, :],
                        lhsT=w_sb[:, ck, n * 64:(n + 1) * 64],
                        rhs=rhs,
                        start=(ck == 0),
                        stop=(ck == CK - 1),
                    )

        # ---------- squash over d (partition dim -> ones-matmul reductions;
        # rsqrt = exp(-0.5 ln(x+eps)) keeps ACT on one table set) ----------
        with nc.named_scope(f"r{r}_sq"):
            pre_sb = work.tile([64, NB], F32, tag="pre_sb")
            nc.scalar.copy(out=pre_sb[:], in_=pre_ps[:].rearrange("d n b -> d (n b)"))
            sq_sb = work.tile([64, NB], F32, tag="sq")
            nc.vector.tensor_mul(sq_sb[:], pre_sb[:], pre_sb[:])
            ss_ps = ps.tile([1, NB], F32, tag="sqps")
            nc.tensor.matmul(
                out=ss_ps[:], lhsT=ones_col[0:64, :], rhs=sq_sb[:],
                start=True, stop=True,
            )
            # r == 0 squashes pre/N: sum scales by 1/N^2, output by 1/N
            # (folded as exp(-0.5 ln(s/N^2 + eps) + ln(1/N)) * pre)
            ln_sb = work.tile([1, NB], F32, tag="ln")
            nc.scalar.activation(
                out=ln_sb[:], in_=ss_ps[:], func=AF.Ln,
                bias=eps1[:], scale=(1.0 / (N * N) if r == 0 else 1.0),
            )
            rn_sb = work.tile([1, NB], F32, tag="rn")
            nc.scalar.activation(
                out=rn_sb[:], in_=ln_sb[:], func=AF.Exp, scale=-0.5,
                bias=(negln_n[:] if r == 0 else zero1[:]),
            )
            rnb_ps = ps.tile([64, NB], F32, tag="sqps")
            nc.tensor.matmul(
                out=rnb_ps[:], lhsT=ones_row[:], rhs=rn_sb[:],
                start=True, stop=True,
            )
            o_sb = work.tile([64, NB], F32, tag="o")
            nc.vector.tensor_tensor(o_sb[:], pre_sb[:], rnb_ps[:], MULT)

    # ---------- write out: out[b, n, d] = o[d, (n,b)] ----------
    with nc.named_scope("out"):
        ot_ps = ps.tile([128, 64], F32, tag="sqps")
        nc.tensor.transpose(ot_ps[:], o_sb[:], ident[0:64, 0:64])
        ot_sb = work.tile([128, 64], F32, tag="ot")
        nc.scalar.copy(out=ot_sb[:], in_=ot_ps[:])
        out_nbd = bass.AP(
            tensor=out_ap.tensor,
            offset=out_ap.offset,
            ap=[[D, N], [N * D, BL], [1, D]],
        )
        nc.sync.dma_start(out=out_nbd, in_=ot_sb[:])


def build_program():
    nc = bacc.Bacc("TRN2", target_bir_lowering=False, debug=False)
    u_ap = nc.dram_tensor("u", [BL, I, C], F32, kind="ExternalInput").ap()
    w_ap = nc.dram_tensor("w", [C, ND], F32, kind="ExternalInput").ap()
    out_ap = nc.dram_tensor("out", [BL, N, D], F32, kind="ExternalOutput").ap()
    with tile.TileContext(nc) as tc:
        with ExitStack() as ctx:
            _capsule_body(ctx, tc, out_ap, u_ap, w_ap)
    nc.compile()
    return nc


_NC = None


def kernel(u_vecs: np.ndarray, W: np.ndarray) -> np.ndarray:
    global _NC
    u = np.ascontiguousarray(np.asarray(u_vecs, dtype=np.float32))
    w = np.ascontiguousarray(np.asarray(W, dtype=np.float32))
    assert u.shape == (B, I, C) and w.shape == (C, ND)
    if _NC is None:
        _NC = build_program()
    in_maps = [
        {"u": u[i * BL:(i + 1) * BL], "w": w} for i in range(NCORES)
    ]
    res = run_bass_kernel_spmd(_NC, in_maps, list(range(NCORES)))
    return np.concatenate(
        [res.results[i]["out"] for i in range(NCORES)], axis=0
    )
